# revision 18
# baseline (speedup 1.0000x reference)
"""GaborConv2d Trainium2 kernel (u4 supertaps, on-chip shift expansion).

Strategy
--------
Host: generate the tiny [64,3,7,7] Gabor weights from (freq, theta, sigma,
psi), pad the input to [518 rows, 520 cols], and build a compact u=0 plane
stack: for each gb block of G=4 output rows, 30 planes p = 3*ri + c
(ri in 0..9, c in 0..2) hold pad[c, gb*4+ri, :].  Planes are padded to 32
per block (2 zero planes) so tiles stay 32-aligned.

Device (per core, 2 images batch-sharded): tiles of 8 gb blocks (32 output
rows), one block per 520-wide strip, 32 planes in partitions 0..31.  The
DVE builds 3 shifted copies on-chip with two cross-quadrant copies
(partitions 32..63 = shift 1; 64..127 = shift 2 of 0..63), giving planes
(u, ri, c) at partition 32u + 3ri + c for u in 0..3.  With 4 x-shifts
resident in partitions, one output-row pair (M=128 = 2 rows x 64 ch)
finishes in TWO accumulating K=128 matmuls (taps kw = 4t + u).

PE cost model (HW-measured): an N=512 bf16 matmul streams 1 col/cycle at
K>64 (~222 ns) and 2 cols/cycle at K<=64 (~120 ns), row-half streams do
NOT overlap, so a pair costs ~445 ns either as 2xK=128 or 4xK<=64; the
K=128 form halves the instruction count (1024 MMs, ~225 us PE busy,
~92% of the kernel).  Coverage arithmetic rules out 3-chunk packings
(the leftover 3-kw window always needs 72 > 64 planes), and fp8 fails
the 2e-2 budget (5.7e-2 single chain, 2.6e-2 w-corrected).

HBM input traffic drops 15.9 -> 8.5 MB vs the 42-plane im2col (shifted
copies never touch HBM); output is bf16 staged [128, 16*512] and stored
as two 1 MB sequential transfers per tile (8 KB per-partition
descriptors).  PSUM evictions (f32->bf16 casts) split 6:10 DVE:ScalarE
(whole banks only -- two engines on one bank collide on the PSUM read
port); input-load descriptors issue from the otherwise-idle GpSimd
queue so ScalarE's eviction backlog can't starve the prefetch.  12
warm-up matmuls bridge the HAM 1.2->2.4 GHz ramp while tile 0 loads.
Output is upcast f32 on host.  Measured: 244.6-246.9 us HW exec
(vs 270.0 us for the prior 60-plane/4-tap kernel on the same pod).
"""

import math

import ml_dtypes
import numpy as np

import concourse.bass as bass
import concourse.mybir as mybir
import concourse.tile as tile
from concourse import bacc
from concourse.bass_utils import run_bass_kernel_spmd

F32 = mybir.dt.float32
BF16 = mybir.dt.bfloat16
BF16NP = ml_dtypes.bfloat16

N_CORES = 8
B, C, H, W = 16, 3, 512, 512
O, K, PAD = 64, 7, 3
IPC = B // N_CORES          # images per core
G = 4                       # output rows per gb block
NGB = H // G                # 128 gb blocks per image
GBT = 8                     # gb blocks (strips) per SBUF tile (32 rows)
NT = NGB // GBT             # 16 tiles per image
SW = 520                    # stored strip width (512 + 4 tap + 3 shift + pad)
NPL = 32                    # planes per block (30 real + 2 zero)
DELTA = 0.001


def _gabor_weights(freq, theta, sigma, psi):
    x0 = math.ceil(K / 2)
    lin = np.linspace(-x0 + 1, x0, K, dtype=np.float32)
    y = np.broadcast_to(lin[:, None], (K, K))
    x = np.broadcast_to(lin[None, :], (K, K))
    th = theta[:, :, None, None].astype(np.float32)
    fr = freq[:, :, None, None].astype(np.float32)
    sg = sigma[:, :, None, None].astype(np.float32)
    ps = psi[:, :, None, None].astype(np.float32)
    rotx = x * np.cos(th) + y * np.sin(th)
    roty = -x * np.sin(th) + y * np.cos(th)
    g = np.exp(-0.5 * ((rotx**2 + roty**2) / (sg + DELTA) ** 2))
    g = g * np.cos(fr * rotx + ps)
    g = g / (2 * np.pi * sg**2)
    return g.astype(np.float32)  # [O, C, K, K]


def _build_nc():
    nc = bacc.Bacc(None, target_bir_lowering=False)
    # xstack[img, tl, p, s*SW + x]: u=0 plane p = 3ri+c of block tl*8 + s
    xs = nc.dram_tensor("xstack", [IPC, NT, NPL, GBT * SW], BF16, kind="ExternalInput")
    # wbig[32u + 3ri + c, (2jp+t)*128 + 64dr + o] = g[o, c, ri-(2jp+dr), 4t+u]
    wb = nc.dram_tensor("wbig", [128, 4 * 128], BF16, kind="ExternalInput")
    # ydev[img, tl, 64dr + o, (2s+jp)*512 + x] = out[img, o, 32tl+4s+2jp+dr, x]
    y = nc.dram_tensor("y", [IPC, NT, 128, 16 * W], BF16, kind="ExternalOutput")
    junk = nc.dram_tensor("junk", [128, 4], F32, kind="ExternalOutput")

    with tile.TileContext(nc) as tc:
        with (
            tc.tile_pool(name="wpool", bufs=1) as wpool,
            tc.tile_pool(name="ipool", bufs=6) as ipool,
            tc.tile_pool(name="spool", bufs=3) as spool,
            tc.tile_pool(name="ppool", bufs=8, space="PSUM") as ppool,
        ):
            # tile 0's input load goes out first on the sync queue, which
            # starts ~2.5us before the gpsimd queue clears its preamble
            it0 = ipool.tile([128, GBT, SW], BF16, tag="img")
            nc.sync.dma_start(
                out=it0[0:NPL, :, :],
                in_=bass.AP(xs, 0, [[GBT * SW, NPL], [1, GBT * SW]]),
            )
            wt = wpool.tile([128, 4 * 128], BF16)
            nc.sync.dma_start(out=wt, in_=wb[:])

            # HAM warm-up: ~3.5us of dense matmuls so the PE clock is at
            # 2.4GHz when the real stream starts; sized to end right as
            # tile 0's expansion completes.
            dps = ppool.tile([128, W], F32, tag="ps")
            for wi in range(12):
                nc.tensor.matmul(
                    dps[:, :],
                    wt[:, 0:128],
                    wt[:, 0:W],
                    start=(wi == 0),
                    stop=(wi == 11),
                )
            # junk eviction on ScalarE: keeps the DVE FIFO free so tile 0's
            # expansion copies don't queue behind the warmup chain
            jt = wpool.tile([128, 4], F32)
            nc.scalar.copy(jt, dps[:, 0:4])
            nc.sync.dma_start(out=junk[:], in_=jt)

            pidx = 0
            for img in range(IPC):
                for tl in range(NT):
                    if img == 0 and tl == 0:
                        it = it0
                    else:
                        it = ipool.tile([128, GBT, SW], BF16, tag="img")
                        # issue loads from the otherwise-idle GpSimd queue --
                        # ScalarE is busy with evictions and would delay the
                        # descriptor issue, starving the prefetch pipeline
                        nc.gpsimd.dma_start(
                            out=it[0:NPL, :, :],
                            in_=bass.AP(
                                xs,
                                (img * NT + tl) * NPL * GBT * SW,
                                [[GBT * SW, NPL], [1, GBT * SW]],
                            ),
                        )
                    # on-chip shift expansion (DVE cross-quadrant copies):
                    # partitions 32..63 = shift-1, 64..127 = shift-2 of 0..63.
                    # op1 writes [0:518) so op2's src reads [2:518) stay
                    # initialized (matmuls only read [0:516) of any plane).
                    nc.vector.tensor_copy(
                        it[32:64, :, 0:518], it[0:32, :, 1:519]
                    )
                    nc.vector.tensor_copy(
                        it[64:128, :, 0:516], it[0:64, :, 2:518]
                    )
                    stg = spool.tile([128, 16 * W], BF16, tag="stg")
                    for s in range(GBT):
                        for jp in range(2):
                            ps = ppool.tile([128, W], F32, tag="ps")
                            for t in range(2):
                                nc.tensor.matmul(
                                    ps[:, :],
                                    wt[:, (2 * jp + t) * 128 : (2 * jp + t) * 128 + 128],
                                    it[0:128, s, 4 * t : 4 * t + W],
                                    start=(t == 0),
                                    stop=(t == 1),
                                )
                            slot = 2 * s + jp
                            sl = stg[:, slot * W : (slot + 1) * W]
                            # whole-psum evictions (two engines on one bank
                            # conflict on the PSUM read port); DVE also runs
                            # the expansion copies so it gets a smaller share
                            last = img == IPC - 1 and tl == NT - 1
                            # last tile: balance 8/8 so both engines finish
                            # the tail together
                            dve_share = 8 if last else 6
                            if pidx % 16 < dve_share:
                                nc.vector.tensor_copy(sl, ps[:, :])
                            else:
                                nc.scalar.copy(sl, ps[:, :])
                            pidx += 1
                    # split stores (8KB descriptors) so earlier chunks
                    # stream out while the last evictions finish; quarter
                    # the final tile to shorten the kernel tail
                    nhf = 4 if img == IPC - 1 and tl == NT - 1 else 2
                    for hf in range(nhf):
                        q = 16 // nhf
                        nc.sync.dma_start(
                            out=bass.AP(
                                y,
                                (img * NT + tl) * 128 * 16 * W + hf * q * W,
                                [[16 * W, 128], [1, q * W]],
                            ),
                            in_=stg[:, hf * q * W : (hf + 1) * q * W],
                        )
    nc.finalize()
    return nc


def _prepare_inputs(input_tensor, freq, theta, sigma, psi):
    g = _gabor_weights(freq, theta, sigma, psi)  # [O, C, K, K] f32
    wmat = np.zeros((128, 4 * 128), np.float32)
    for u in range(4):
        for ri in range(10):
            for c in range(C):
                p = 32 * u + 3 * ri + c
                for jp in range(2):
                    for t in range(2):
                        kj = 4 * t + u
                        if kj >= K:
                            continue
                        for dr in range(2):
                            kr = ri - (2 * jp + dr)
                            if 0 <= kr < K:
                                col = (2 * jp + t) * 128 + 64 * dr
                                wmat[p, col : col + O] = g[:, c, kr, kj]
    wbig = np.ascontiguousarray(wmat).astype(BF16NP)

    xb = np.asarray(input_tensor, dtype=np.float32).astype(BF16NP)
    pad = np.zeros((B, C, H + 2 * PAD, SW), BF16NP)
    pad[:, :, PAD : PAD + H, PAD : PAD + W] = xb
    # xstack[img, tl, 3ri+c, s, x] = pad[img, c, (8tl+s)*4 + ri, x]
    xstack = np.zeros((B, NT, NPL, GBT, SW), BF16NP)
    for ri in range(10):
        for c in range(C):
            xstack[:, :, 3 * ri + c, :, :] = pad[:, c, ri : ri + H : G, :].reshape(
                B, NT, GBT, SW
            )
    xstack = np.ascontiguousarray(xstack.reshape(B, NT, NPL, GBT * SW))
    in_maps = [
        {"xstack": xstack[core * IPC : (core + 1) * IPC], "wbig": wbig}
        for core in range(N_CORES)
    ]
    return in_maps


_NC_CACHE = None


def kernel(input_tensor, freq, theta, sigma, psi):
    global _NC_CACHE
    input_tensor = np.asarray(input_tensor, dtype=np.float32)
    in_maps = _prepare_inputs(
        input_tensor,
        np.asarray(freq), np.asarray(theta), np.asarray(sigma), np.asarray(psi),
    )
    if _NC_CACHE is None:
        _NC_CACHE = _build_nc()
    res = run_bass_kernel_spmd(_NC_CACHE, in_maps, core_ids=list(range(N_CORES)))
    out = np.concatenate([r["y"] for r in res.results], axis=0)
    # ydev[img, tl, 64dr+o, (2s+jp)*512+x] -> y[img, o, 32tl+4s+2jp+dr, x]
    out = (
        out.reshape(B, NT, 2, O, GBT, 2, W)
        .transpose(0, 3, 1, 4, 5, 2, 6)
        .reshape(B, O, H, W)
    )
    return out.astype(np.float32)


# revision 21
# speedup vs baseline: 1.0111x; 1.0111x over previous
"""GaborConv2d Trainium2 kernel (u4 supertaps, on-chip shift expansion).

Strategy
--------
Host: generate the tiny [64,3,7,7] Gabor weights from (freq, theta, sigma,
psi), pad the input to [518 rows, 520 cols], and build a compact u=0 plane
stack: for each gb block of G=4 output rows, 30 planes p = 3*ri + c
(ri in 0..9, c in 0..2) hold pad[c, gb*4+ri, :].  Planes are padded to 32
per block (2 zero planes) so tiles stay 32-aligned.

Device (per core, 2 images batch-sharded): tiles of 8 gb blocks (32 output
rows), one block per 520-wide strip, 32 planes in partitions 0..31.  The
DVE builds 3 shifted copies on-chip with two cross-quadrant copies
(partitions 32..63 = shift 1; 64..127 = shift 2 of 0..63), giving planes
(u, ri, c) at partition 32u + 3ri + c for u in 0..3.  With 4 x-shifts
resident in partitions, one output-row pair (M=128 = 2 rows x 64 ch)
finishes in TWO accumulating K=128 matmuls (taps kw = 4t + u).

PE cost model (HW-measured): an N=512 bf16 matmul streams 1 col/cycle at
K>64 (~222 ns) and 2 cols/cycle at K<=64 (~120 ns), row-half streams do
NOT overlap, so a pair costs ~445 ns either as 2xK=128 or 4xK<=64; the
K=128 form halves the instruction count (1024 MMs, ~225 us PE busy,
~92% of the kernel).  Coverage arithmetic rules out 3-chunk packings
(the leftover 3-kw window always needs 72 > 64 planes), and fp8 fails
the 2e-2 budget (5.7e-2 single chain, 2.6e-2 w-corrected).

HBM input traffic drops 15.9 -> 8.5 MB vs the 42-plane im2col (shifted
copies never touch HBM); output is bf16 staged [128, 16*512] and stored
as two 1 MB sequential transfers per tile (8 KB per-partition
descriptors).  PSUM evictions (f32->bf16 casts) split 6:10 DVE:ScalarE
(whole banks only -- two engines on one bank collide on the PSUM read
port); input-load descriptors issue from the otherwise-idle GpSimd
queue so ScalarE's eviction backlog can't starve the prefetch.  12
warm-up matmuls bridge the HAM 1.2->2.4 GHz ramp while tile 0 loads.
Output is upcast f32 on host.  Measured: 244.6-246.9 us HW exec
(vs 270.0 us for the prior 60-plane/4-tap kernel on the same pod).
"""

import math

import ml_dtypes
import numpy as np

import concourse.bass as bass
import concourse.mybir as mybir
import concourse.tile as tile
from concourse import bacc
from concourse.bass_utils import run_bass_kernel_spmd

F32 = mybir.dt.float32
BF16 = mybir.dt.bfloat16
BF16NP = ml_dtypes.bfloat16

N_CORES = 8
B, C, H, W = 16, 3, 512, 512
O, K, PAD = 64, 7, 3
IPC = B // N_CORES          # images per core
G = 4                       # output rows per gb block
NGB = H // G                # 128 gb blocks per image
GBT = 8                     # gb blocks (strips) per SBUF tile (32 rows)
NT = NGB // GBT             # 16 tiles per image
SW = 520                    # stored strip width (512 + 4 tap + 3 shift + pad)
NPL = 32                    # planes per block (30 real + 2 zero)
DELTA = 0.001


def _gabor_weights(freq, theta, sigma, psi):
    x0 = math.ceil(K / 2)
    lin = np.linspace(-x0 + 1, x0, K, dtype=np.float32)
    y = np.broadcast_to(lin[:, None], (K, K))
    x = np.broadcast_to(lin[None, :], (K, K))
    th = theta[:, :, None, None].astype(np.float32)
    fr = freq[:, :, None, None].astype(np.float32)
    sg = sigma[:, :, None, None].astype(np.float32)
    ps = psi[:, :, None, None].astype(np.float32)
    rotx = x * np.cos(th) + y * np.sin(th)
    roty = -x * np.sin(th) + y * np.cos(th)
    g = np.exp(-0.5 * ((rotx**2 + roty**2) / (sg + DELTA) ** 2))
    g = g * np.cos(fr * rotx + ps)
    g = g / (2 * np.pi * sg**2)
    return g.astype(np.float32)  # [O, C, K, K]


def _build_nc():
    nc = bacc.Bacc(None, target_bir_lowering=False)
    # xstack[img, tl, p, s*SW + x]: u=0 plane p = 3ri+c of block tl*8 + s
    xs = nc.dram_tensor("xstack", [IPC, NT, NPL, GBT * SW], BF16, kind="ExternalInput")
    # wbig[32u + 3ri + c, (2jp+t)*128 + 64dr + o] = g[o, c, ri-(2jp+dr), 4t+u]
    wb = nc.dram_tensor("wbig", [128, 4 * 128], BF16, kind="ExternalInput")
    # ydev[img, tl, 64dr + o, (2s+jp)*512 + x] = out[img, o, 32tl+4s+2jp+dr, x]
    y = nc.dram_tensor("y", [IPC, NT, 128, 16 * W], BF16, kind="ExternalOutput")
    junk = nc.dram_tensor("junk", [128, 4], F32, kind="ExternalOutput")

    with tile.TileContext(nc) as tc:
        with (
            tc.tile_pool(name="wpool", bufs=1) as wpool,
            tc.tile_pool(name="ipool", bufs=6) as ipool,
            tc.tile_pool(name="spool", bufs=3) as spool,
            tc.tile_pool(name="ppool", bufs=8, space="PSUM") as ppool,
        ):
            wt = wpool.tile([128, 4 * 128], BF16)
            nc.sync.dma_start(out=wt, in_=wb[:])

            # HAM warm-up: ~3.5us of dense matmuls so the PE clock is at
            # 2.4GHz when the real stream starts; sized to end right as
            # tile 0's expansion completes.
            dps = ppool.tile([128, W], F32, tag="ps")
            for wi in range(12):
                nc.tensor.matmul(
                    dps[:, :],
                    wt[:, 0:128],
                    wt[:, 0:W],
                    start=(wi == 0),
                    stop=(wi == 11),
                )
            # junk eviction on ScalarE: keeps the DVE FIFO free so tile 0's
            # expansion copies don't queue behind the warmup chain
            jt = wpool.tile([128, 4], F32)
            nc.scalar.copy(jt, dps[:, 0:4])
            nc.sync.dma_start(out=junk[:], in_=jt)

            pidx = 0
            for img in range(IPC):
                for tl in range(NT):
                    it = ipool.tile([128, GBT, SW], BF16, tag="img")
                    # issue loads from the otherwise-idle GpSimd queue --
                    # ScalarE is busy with evictions and would delay the
                    # descriptor issue, starving the prefetch pipeline
                    nc.gpsimd.dma_start(
                        out=it[0:NPL, :, :],
                        in_=bass.AP(
                            xs,
                            (img * NT + tl) * NPL * GBT * SW,
                            [[GBT * SW, NPL], [1, GBT * SW]],
                        ),
                    )
                    # on-chip shift expansion (DVE cross-quadrant copies):
                    # partitions 32..63 = shift-1, 64..127 = shift-2 of 0..63.
                    # op1 writes [0:518) so op2's src reads [2:518) stay
                    # initialized (matmuls only read [0:516) of any plane).
                    # Tile 0 expands per strip-half so its first matmuls
                    # start ~1.2us sooner.
                    if img == 0 and tl == 0:
                        for sh in range(2):
                            hb, he = sh * 4, sh * 4 + 4
                            nc.vector.tensor_copy(
                                it[32:64, hb:he, 0:518], it[0:32, hb:he, 1:519]
                            )
                            nc.vector.tensor_copy(
                                it[64:128, hb:he, 0:516], it[0:64, hb:he, 2:518]
                            )
                    else:
                        nc.vector.tensor_copy(
                            it[32:64, :, 0:518], it[0:32, :, 1:519]
                        )
                        nc.vector.tensor_copy(
                            it[64:128, :, 0:516], it[0:64, :, 2:518]
                        )
                    stg = spool.tile([128, 16 * W], BF16, tag="stg")
                    for s in range(GBT):
                        for jp in range(2):
                            ps = ppool.tile([128, W], F32, tag="ps")
                            for t in range(2):
                                nc.tensor.matmul(
                                    ps[:, :],
                                    wt[:, (2 * jp + t) * 128 : (2 * jp + t) * 128 + 128],
                                    it[0:128, s, 4 * t : 4 * t + W],
                                    start=(t == 0),
                                    stop=(t == 1),
                                )
                            slot = 2 * s + jp
                            sl = stg[:, slot * W : (slot + 1) * W]
                            # whole-psum evictions (two engines on one bank
                            # conflict on the PSUM read port); DVE also runs
                            # the expansion copies so it gets a smaller share
                            last = img == IPC - 1 and tl == NT - 1
                            # last tile: balance 8/8 so both engines finish
                            # the tail together
                            dve_share = 8 if last else 6
                            if pidx % 16 < dve_share:
                                nc.vector.tensor_copy(sl, ps[:, :])
                            else:
                                nc.scalar.copy(sl, ps[:, :])
                            pidx += 1
                    # split stores (8KB descriptors) so earlier chunks
                    # stream out while the last evictions finish; eighth
                    # the final tile to shorten the kernel tail
                    nhf = 8 if img == IPC - 1 and tl == NT - 1 else 2
                    for hf in range(nhf):
                        q = 16 // nhf
                        nc.sync.dma_start(
                            out=bass.AP(
                                y,
                                (img * NT + tl) * 128 * 16 * W + hf * q * W,
                                [[16 * W, 128], [1, q * W]],
                            ),
                            in_=stg[:, hf * q * W : (hf + 1) * q * W],
                        )
    nc.finalize()
    return nc


def _prepare_inputs(input_tensor, freq, theta, sigma, psi):
    g = _gabor_weights(freq, theta, sigma, psi)  # [O, C, K, K] f32
    wmat = np.zeros((128, 4 * 128), np.float32)
    for u in range(4):
        for ri in range(10):
            for c in range(C):
                p = 32 * u + 3 * ri + c
                for jp in range(2):
                    for t in range(2):
                        kj = 4 * t + u
                        if kj >= K:
                            continue
                        for dr in range(2):
                            kr = ri - (2 * jp + dr)
                            if 0 <= kr < K:
                                col = (2 * jp + t) * 128 + 64 * dr
                                wmat[p, col : col + O] = g[:, c, kr, kj]
    wbig = np.ascontiguousarray(wmat).astype(BF16NP)

    xb = np.asarray(input_tensor, dtype=np.float32).astype(BF16NP)
    pad = np.zeros((B, C, H + 2 * PAD, SW), BF16NP)
    pad[:, :, PAD : PAD + H, PAD : PAD + W] = xb
    # xstack[img, tl, 3ri+c, s, x] = pad[img, c, (8tl+s)*4 + ri, x]
    xstack = np.zeros((B, NT, NPL, GBT, SW), BF16NP)
    for ri in range(10):
        for c in range(C):
            xstack[:, :, 3 * ri + c, :, :] = pad[:, c, ri : ri + H : G, :].reshape(
                B, NT, GBT, SW
            )
    xstack = np.ascontiguousarray(xstack.reshape(B, NT, NPL, GBT * SW))
    in_maps = [
        {"xstack": xstack[core * IPC : (core + 1) * IPC], "wbig": wbig}
        for core in range(N_CORES)
    ]
    return in_maps


_NC_CACHE = None


def kernel(input_tensor, freq, theta, sigma, psi):
    global _NC_CACHE
    input_tensor = np.asarray(input_tensor, dtype=np.float32)
    in_maps = _prepare_inputs(
        input_tensor,
        np.asarray(freq), np.asarray(theta), np.asarray(sigma), np.asarray(psi),
    )
    if _NC_CACHE is None:
        _NC_CACHE = _build_nc()
    res = run_bass_kernel_spmd(_NC_CACHE, in_maps, core_ids=list(range(N_CORES)))
    out = np.concatenate([r["y"] for r in res.results], axis=0)
    # ydev[img, tl, 64dr+o, (2s+jp)*512+x] -> y[img, o, 32tl+4s+2jp+dr, x]
    out = (
        out.reshape(B, NT, 2, O, GBT, 2, W)
        .transpose(0, 3, 1, 4, 5, 2, 6)
        .reshape(B, O, H, W)
    )
    return out.astype(np.float32)


# revision 22
# speedup vs baseline: 1.0234x; 1.0122x over previous
"""GaborConv2d Trainium2 kernel (u4 supertaps, on-chip shift expansion).

Strategy
--------
Host: generate the tiny [64,3,7,7] Gabor weights from (freq, theta, sigma,
psi), pad the input to [518 rows, 520 cols], and build a compact u=0 plane
stack: for each gb block of G=4 output rows, 30 planes p = 3*ri + c
(ri in 0..9, c in 0..2) hold pad[c, gb*4+ri, :].  Planes are padded to 32
per block (2 zero planes) so tiles stay 32-aligned.

Device (per core, 2 images batch-sharded): tiles of 8 gb blocks (32 output
rows), one block per 520-wide strip, 32 planes in partitions 0..31.  The
DVE builds 3 shifted copies on-chip with two cross-quadrant copies
(partitions 32..63 = shift 1; 64..127 = shift 2 of 0..63), giving planes
(u, ri, c) at partition 32u + 3ri + c for u in 0..3.  With 4 x-shifts
resident in partitions, one output-row pair (M=128 = 2 rows x 64 ch)
finishes in TWO accumulating K=128 matmuls (taps kw = 4t + u).

PE cost model (HW-measured): an N=512 bf16 matmul streams 1 col/cycle at
K>64 (~222 ns) and 2 cols/cycle at K<=64 (~120 ns), row-half streams do
NOT overlap, so a pair costs ~445 ns either as 2xK=128 or 4xK<=64; the
K=128 form halves the instruction count (1024 MMs, ~225 us PE busy,
~92% of the kernel).  Coverage arithmetic rules out 3-chunk packings
(the leftover 3-kw window always needs 72 > 64 planes), and fp8 fails
the 2e-2 budget (5.7e-2 single chain, 2.6e-2 w-corrected).

HBM input traffic drops 15.9 -> 8.5 MB vs the 42-plane im2col (shifted
copies never touch HBM); output is bf16 staged [128, 16*512] and stored
as two 1 MB sequential transfers per tile (8 KB per-partition
descriptors).  PSUM evictions (f32->bf16 casts) split 6:10 DVE:ScalarE
(whole banks only -- two engines on one bank collide on the PSUM read
port); input-load descriptors issue from the otherwise-idle GpSimd
queue so ScalarE's eviction backlog can't starve the prefetch.  12
warm-up matmuls bridge the HAM 1.2->2.4 GHz ramp while tile 0 loads.
Output is upcast f32 on host.  Measured: 244.6-246.9 us HW exec
(vs 270.0 us for the prior 60-plane/4-tap kernel on the same pod).
"""

import math

import ml_dtypes
import numpy as np

import concourse.bass as bass
import concourse.mybir as mybir
import concourse.tile as tile
from concourse import bacc
from concourse.bass_utils import run_bass_kernel_spmd

F32 = mybir.dt.float32
BF16 = mybir.dt.bfloat16
BF16NP = ml_dtypes.bfloat16

N_CORES = 8
B, C, H, W = 16, 3, 512, 512
O, K, PAD = 64, 7, 3
IPC = B // N_CORES          # images per core
G = 4                       # output rows per gb block
NGB = H // G                # 128 gb blocks per image
GBT = 8                     # gb blocks (strips) per SBUF tile (32 rows)
NT = NGB // GBT             # 16 tiles per image
SW = 520                    # stored strip width (512 + 4 tap + 3 shift + pad)
NPL = 32                    # planes per block (30 real + 2 zero)
DELTA = 0.001


def _gabor_weights(freq, theta, sigma, psi):
    x0 = math.ceil(K / 2)
    lin = np.linspace(-x0 + 1, x0, K, dtype=np.float32)
    y = np.broadcast_to(lin[:, None], (K, K))
    x = np.broadcast_to(lin[None, :], (K, K))
    th = theta[:, :, None, None].astype(np.float32)
    fr = freq[:, :, None, None].astype(np.float32)
    sg = sigma[:, :, None, None].astype(np.float32)
    ps = psi[:, :, None, None].astype(np.float32)
    rotx = x * np.cos(th) + y * np.sin(th)
    roty = -x * np.sin(th) + y * np.cos(th)
    g = np.exp(-0.5 * ((rotx**2 + roty**2) / (sg + DELTA) ** 2))
    g = g * np.cos(fr * rotx + ps)
    g = g / (2 * np.pi * sg**2)
    return g.astype(np.float32)  # [O, C, K, K]


def _build_nc():
    nc = bacc.Bacc(None, target_bir_lowering=False)
    # xstack[img, tl, p, s*SW + x]: u=0 plane p = 3ri+c of block tl*8 + s
    xs = nc.dram_tensor("xstack", [IPC, NT, NPL, GBT * SW], BF16, kind="ExternalInput")
    # wbig[32u + 3ri + c, (2jp+t)*128 + 64dr + o] = g[o, c, ri-(2jp+dr), 4t+u]
    wb = nc.dram_tensor("wbig", [128, 4 * 128], BF16, kind="ExternalInput")
    # ydev[img, tl, 64dr + o, (2s+jp)*512 + x] = out[img, o, 32tl+4s+2jp+dr, x]
    y = nc.dram_tensor("y", [IPC, NT, 128, 16 * W], BF16, kind="ExternalOutput")
    junk = nc.dram_tensor("junk", [128, 4], F32, kind="ExternalOutput")

    with tile.TileContext(nc) as tc:
        with (
            tc.tile_pool(name="wpool", bufs=1) as wpool,
            tc.tile_pool(name="ipool", bufs=6) as ipool,
            tc.tile_pool(name="spool", bufs=3) as spool,
            tc.tile_pool(name="ppool", bufs=8, space="PSUM") as ppool,
        ):
            wt = wpool.tile([128, 4 * 128], BF16)
            nc.sync.dma_start(out=wt, in_=wb[:])

            # HAM warm-up: ~3.5us of dense matmuls so the PE clock is at
            # 2.4GHz when the real stream starts; sized to end right as
            # tile 0's expansion completes.
            dps = ppool.tile([128, W], F32, tag="ps")
            for wi in range(12):
                nc.tensor.matmul(
                    dps[:, :],
                    wt[:, 0:128],
                    wt[:, 0:W],
                    start=(wi == 0),
                    stop=(wi == 11),
                )
            # junk eviction on ScalarE: keeps the DVE FIFO free so tile 0's
            # expansion copies don't queue behind the warmup chain
            jt = wpool.tile([128, 4], F32)
            nc.scalar.copy(jt, dps[:, 0:4])
            nc.sync.dma_start(out=junk[:], in_=jt)

            pidx = 0
            for img in range(IPC):
                for tl in range(NT):
                    it = ipool.tile([128, GBT, SW], BF16, tag="img")
                    # issue loads from the otherwise-idle GpSimd queue --
                    # ScalarE is busy with evictions and would delay the
                    # descriptor issue, starving the prefetch pipeline
                    nc.gpsimd.dma_start(
                        out=it[0:NPL, :, :],
                        in_=bass.AP(
                            xs,
                            (img * NT + tl) * NPL * GBT * SW,
                            [[GBT * SW, NPL], [1, GBT * SW]],
                        ),
                    )
                    # on-chip shift expansion (DVE cross-quadrant copies):
                    # partitions 32..63 = shift-1, 64..127 = shift-2 of 0..63.
                    # op1 writes [0:518) so op2's src reads [2:518) stay
                    # initialized (matmuls only read [0:516) of any plane).
                    # Tile 0 expands per strip-half so its first matmuls
                    # start ~1.2us sooner.
                    if img == 0 and tl == 0:
                        for sh in range(2):
                            hb, he = sh * 4, sh * 4 + 4
                            nc.vector.tensor_copy(
                                it[32:64, hb:he, 0:518], it[0:32, hb:he, 1:519]
                            )
                            nc.vector.tensor_copy(
                                it[64:128, hb:he, 0:516], it[0:64, hb:he, 2:518]
                            )
                    else:
                        nc.vector.tensor_copy(
                            it[32:64, :, 0:518], it[0:32, :, 1:519]
                        )
                        nc.vector.tensor_copy(
                            it[64:128, :, 0:516], it[0:64, :, 2:518]
                        )
                    stg = spool.tile([128, 16 * W], BF16, tag="stg")
                    for s in range(GBT):
                        for jp in range(2):
                            ps = ppool.tile([128, W], F32, tag="ps")
                            for t in range(2):
                                nc.tensor.matmul(
                                    ps[:, :],
                                    wt[:, (2 * jp + t) * 128 : (2 * jp + t) * 128 + 128],
                                    it[0:128, s, 4 * t : 4 * t + W],
                                    start=(t == 0),
                                    stop=(t == 1),
                                )
                            slot = 2 * s + jp
                            sl = stg[:, slot * W : (slot + 1) * W]
                            # whole-psum evictions (two engines on one bank
                            # conflict on the PSUM read port); DVE also runs
                            # the expansion copies so it gets a smaller share
                            last = img == IPC - 1 and tl == NT - 1
                            # last tile: balance 8/8 so both engines finish
                            # the tail together
                            dve_share = 8 if last else 6
                            if pidx % 16 < dve_share:
                                nc.vector.tensor_copy(sl, ps[:, :])
                            else:
                                nc.scalar.copy(sl, ps[:, :])
                            pidx += 1
                    # split stores (8KB descriptors) so earlier chunks
                    # stream out while the last evictions finish; quarter
                    # the final tile to shorten the kernel tail
                    nhf = 4 if img == IPC - 1 and tl == NT - 1 else 2
                    for hf in range(nhf):
                        q = 16 // nhf
                        nc.sync.dma_start(
                            out=bass.AP(
                                y,
                                (img * NT + tl) * 128 * 16 * W + hf * q * W,
                                [[16 * W, 128], [1, q * W]],
                            ),
                            in_=stg[:, hf * q * W : (hf + 1) * q * W],
                        )
    nc.finalize()
    return nc


def _prepare_inputs(input_tensor, freq, theta, sigma, psi):
    g = _gabor_weights(freq, theta, sigma, psi)  # [O, C, K, K] f32
    wmat = np.zeros((128, 4 * 128), np.float32)
    for u in range(4):
        for ri in range(10):
            for c in range(C):
                p = 32 * u + 3 * ri + c
                for jp in range(2):
                    for t in range(2):
                        kj = 4 * t + u
                        if kj >= K:
                            continue
                        for dr in range(2):
                            kr = ri - (2 * jp + dr)
                            if 0 <= kr < K:
                                col = (2 * jp + t) * 128 + 64 * dr
                                wmat[p, col : col + O] = g[:, c, kr, kj]
    wbig = np.ascontiguousarray(wmat).astype(BF16NP)

    xb = np.asarray(input_tensor, dtype=np.float32).astype(BF16NP)
    pad = np.zeros((B, C, H + 2 * PAD, SW), BF16NP)
    pad[:, :, PAD : PAD + H, PAD : PAD + W] = xb
    # xstack[img, tl, 3ri+c, s, x] = pad[img, c, (8tl+s)*4 + ri, x]
    xstack = np.zeros((B, NT, NPL, GBT, SW), BF16NP)
    for ri in range(10):
        for c in range(C):
            xstack[:, :, 3 * ri + c, :, :] = pad[:, c, ri : ri + H : G, :].reshape(
                B, NT, GBT, SW
            )
    xstack = np.ascontiguousarray(xstack.reshape(B, NT, NPL, GBT * SW))
    in_maps = [
        {"xstack": xstack[core * IPC : (core + 1) * IPC], "wbig": wbig}
        for core in range(N_CORES)
    ]
    return in_maps


_NC_CACHE = None


def kernel(input_tensor, freq, theta, sigma, psi):
    global _NC_CACHE
    input_tensor = np.asarray(input_tensor, dtype=np.float32)
    in_maps = _prepare_inputs(
        input_tensor,
        np.asarray(freq), np.asarray(theta), np.asarray(sigma), np.asarray(psi),
    )
    if _NC_CACHE is None:
        _NC_CACHE = _build_nc()
    res = run_bass_kernel_spmd(_NC_CACHE, in_maps, core_ids=list(range(N_CORES)))
    out = np.concatenate([r["y"] for r in res.results], axis=0)
    # ydev[img, tl, 64dr+o, (2s+jp)*512+x] -> y[img, o, 32tl+4s+2jp+dr, x]
    out = (
        out.reshape(B, NT, 2, O, GBT, 2, W)
        .transpose(0, 3, 1, 4, 5, 2, 6)
        .reshape(B, O, H, W)
    )
    return out.astype(np.float32)
